# revision 10
# baseline (speedup 1.0000x reference)
"""Trainium2 Bass kernel for nn_Direction: out = input @ qr(weight + 1e-8).Q^T.

input: (262144, 20) fp32, weight: (512, 20) fp32 -> out: (262144, 512) fp32.

Strategy (data-parallel over batch, 8 cores; memory-bound target):
  - Host: QR of the tiny 512x20 weight (LAPACK). Q^T and the input are each
    split into an fp16 hi/lo pair (hi = fp16(v), lo = fp16(v - hi)); the
    matmul is computed as hi@qhi + hi@qlo + lo@qhi on the PE in three fp16
    passes accumulated in fp32 PSUM (~2e-7 rel err, 4x faster than fp32
    matmuls which microcode as two half-rate LOW/HIGH passes).
  - Device per core (32768 rows): 8 slabs of 4096 rows. Rows are padded
    m: 20->32 on host so a single 128x128 PE transpose yields four 32-row
    "row groups"; the three fp16 passes are emitted as term-major waves of
    four row-tiled matmuls (tile_position=(32k,0)) that stream concurrently
    through the PE array.
  - PSUM results are copied (DVE/ACT alternating) into a [128, 32*512] fp32
    SBUF slab whose flat layout equals 4096 consecutive output rows, flushed
    as one contiguous 8MB DMA. The (s, p, j) -> row map (4096 s + 32 p + j)
    is lexicographic, so host-side reshape is free.
"""

import numpy as np

B = 262144
M = 20
MP = 32                    # m padded to 32 for row-group alignment
F = 512
NCORES = 8
BL = B // NCORES           # 32768 rows per core
SLABS = 16
SLAB_ROWS = BL // SLABS    # 2048
CHUNKS = SLAB_ROWS // 128  # 16 chunks of 128 rows per slab
GROUP = 4                  # chunks per transpose group (4*32 = 128 partitions)
NG = CHUNKS // GROUP       # 8 groups per slab

_CACHE = {}


def _build_nc():
    import concourse.bass as bass
    import concourse.tile as tile
    from concourse import bacc, mybir

    f32 = mybir.dt.float32
    f16 = mybir.dt.float16
    COPY = mybir.ActivationFunctionType.Copy

    nc = bacc.Bacc(None, target_bir_lowering=False, debug=False)
    xh = nc.dram_tensor("xh", [SLABS, 128, CHUNKS * MP], f16, kind="ExternalInput")
    xl = nc.dram_tensor("xl", [SLABS, 128, CHUNKS * MP], f16, kind="ExternalInput")
    qh = nc.dram_tensor("qh", [128, F], f16, kind="ExternalInput")
    ql = nc.dram_tensor("ql", [128, F], f16, kind="ExternalInput")
    ident = nc.dram_tensor("ident", [128, 128], f16, kind="ExternalInput")
    out = nc.dram_tensor("out", [SLABS, 128, CHUNKS * F], f32, kind="ExternalOutput")

    with tile.TileContext(nc) as tc:
        with (
            tc.tile_pool(name="const", bufs=1) as cpool,
            tc.tile_pool(name="xin", bufs=3) as xin_pool,
            tc.tile_pool(name="osl", bufs=3) as out_pool,
            tc.tile_pool(name="tt", bufs=3) as tt_pool,
            tc.tile_pool(name="pst", bufs=2, space=bass.MemorySpace.PSUM) as pst_pool,
            tc.tile_pool(name="pso", bufs=6, space=bass.MemorySpace.PSUM) as pso_pool,
        ):
            qh_t = cpool.tile([128, F], f16, tag="qh")
            ql_t = cpool.tile([128, F], f16, tag="ql")
            id_t = cpool.tile([128, 128], f16, tag="id")
            nc.sync.dma_start(qh_t[:], qh[:])
            nc.sync.dma_start(ql_t[:], ql[:])
            nc.sync.dma_start(id_t[:], ident[:])

            # software pipeline: waves+copies for group g are emitted while
            # group g+1's transposes run, so PE never stalls on the DVE copy.
            pending = None

            def emit_group(t, os_tile, g, flush, uid):
                pos = [
                    pso_pool.tile([128, F], f32, name=f"po_{uid}_{k}", tag="po")
                    for k in range(GROUP)
                ]
                # term-major waves: consecutive MMs hit different row groups
                # and stream concurrently through the PE array.
                for term in range(3):
                    for k in range(GROUP):
                        sl = slice(32 * k, 32 * k + 32)
                        hi = t[sl, 0:128]
                        lo = t[sl, 128:256]
                        lhs, rhs = [
                            (hi, qh_t[sl, :]),
                            (hi, ql_t[sl, :]),
                            (lo, qh_t[sl, :]),
                        ][term]
                        nc.tensor.matmul(
                            pos[k][:], lhs, rhs,
                            start=(term == 0), stop=(term == 2),
                            tile_position=(32 * k, 0),
                        )
                for k in range(GROUP):
                    j = g * GROUP + k
                    dst = os_tile[:, j * F:(j + 1) * F]
                    if k % 2 == 0:
                        nc.vector.tensor_copy(dst, pos[k][:])
                    else:
                        nc.scalar.activation(dst, pos[k][:], COPY)
                if flush is not None:
                    fs, ca, cb = flush
                    nc.sync.dma_start(out[fs][:, ca:cb], os_tile[:, ca:cb])

            for s in range(SLABS):
                xh_s = xin_pool.tile([128, CHUNKS * MP], f16, tag="xh_s")
                xl_s = xin_pool.tile([128, CHUNKS * MP], f16, tag="xl_s")
                nc.scalar.dma_start(xh_s[:], xh[s])
                nc.scalar.dma_start(xl_s[:], xl[s])
                os_tile = out_pool.tile([128, CHUNKS * F], f32, name=f"os_{s}", tag="os")
                # flush granularity: 1 group for slab 0, 2 for slab 1,
                # whole slab otherwise (ramp vs DMA-efficiency trade-off)
                fe = 1 if s == 0 else (2 if s == 1 else NG)
                for g in range(NG):
                    pt = pst_pool.tile([128, 256], f16, name=f"pt_{s}_{g}", tag="pt")
                    csl = slice(g * 128, (g + 1) * 128)
                    nc.tensor.transpose(pt[:, 0:128], xh_s[:, csl], id_t[:])
                    nc.tensor.transpose(pt[:, 128:256], xl_s[:, csl], id_t[:])
                    t = tt_pool.tile([128, 256], f16, name=f"t_{s}_{g}", tag="t")
                    nc.vector.tensor_copy(t[:], pt[:])
                    if pending is not None:
                        emit_group(*pending)
                    flush = None
                    if (g + 1) % fe == 0:
                        flush = (s, (g + 1 - fe) * GROUP * F, (g + 1) * GROUP * F)
                    pending = (t, os_tile, g, flush, f"{s}_{g}")
            emit_group(*pending)

    nc.compile()
    return nc


def _get_nc():
    if "nc" not in _CACHE:
        _CACHE["nc"] = _build_nc()
    return _CACHE["nc"]


def _split_f16(a):
    hi = a.astype(np.float16)
    lo = (a - hi.astype(np.float32)).astype(np.float16)
    return hi, lo


def _prep_inputs(input, weight):
    w = weight.astype(np.float32) + np.float32(1e-8)
    q, _ = np.linalg.qr(w)                      # reduced: (512, 20)
    qt = np.ascontiguousarray(q.T.astype(np.float32))  # (20, 512)
    qpad = np.zeros((MP, F), dtype=np.float32)
    qpad[:M] = qt
    qh16, ql16 = _split_f16(qpad)
    qh_rep = np.ascontiguousarray(np.tile(qh16, (GROUP, 1)))
    ql_rep = np.ascontiguousarray(np.tile(ql16, (GROUP, 1)))
    ident = np.eye(128, dtype=np.float16)

    x = np.ascontiguousarray(input.astype(np.float32))
    xh16, xl16 = _split_f16(x)
    xph = np.zeros((B, MP), dtype=np.float16)
    xph[:, :M] = xh16
    xpl = np.zeros((B, MP), dtype=np.float16)
    xpl[:, :M] = xl16
    xph = xph.reshape(NCORES, SLABS, 128, CHUNKS * MP)
    xpl = xpl.reshape(NCORES, SLABS, 128, CHUNKS * MP)
    return [
        {
            "xh": np.ascontiguousarray(xph[c]),
            "xl": np.ascontiguousarray(xpl[c]),
            "qh": qh_rep,
            "ql": ql_rep,
            "ident": ident,
        }
        for c in range(NCORES)
    ]


def _run(input, weight, trace=False):
    from concourse.bass_utils import run_bass_kernel_spmd

    nc = _get_nc()
    in_maps = _prep_inputs(input, weight)
    res = run_bass_kernel_spmd(nc, in_maps, list(range(NCORES)), trace=trace)
    parts = [r["out"].reshape(BL, F) for r in res.results]
    full = np.concatenate(parts, axis=0)
    return full, res


def kernel(input, weight):
    out, _ = _run(input, weight, trace=False)
    return out


# revision 11
# speedup vs baseline: 1.1480x; 1.1480x over previous
"""Trainium2 Bass kernel for nn_Direction: out = input @ qr(weight + 1e-8).Q^T.

input: (262144, 20) fp32, weight: (512, 20) fp32 -> out: (262144, 512) fp32.

Strategy (data-parallel over batch, 8 cores; memory-bound target):
  - Host: QR of the tiny 512x20 weight (LAPACK). Q^T and the input are each
    split into an fp16 hi/lo pair (hi = fp16(v), lo = fp16(v - hi)); the
    matmul is computed as hi@qhi + hi@qlo + lo@qhi on the PE in three fp16
    passes accumulated in fp32 PSUM (~2e-7 rel err, 4x faster than fp32
    matmuls which microcode as two half-rate LOW/HIGH passes).
  - Device per core (32768 rows): 8 slabs of 4096 rows. Rows are padded
    m: 20->32 on host so a single 128x128 PE transpose yields four 32-row
    "row groups"; the three fp16 passes are emitted as term-major waves of
    four row-tiled matmuls (tile_position=(32k,0)) that stream concurrently
    through the PE array.
  - PSUM results are copied (DVE/ACT alternating) into a [128, 32*512] fp32
    SBUF slab whose flat layout equals 4096 consecutive output rows, flushed
    as one contiguous 8MB DMA. The (s, p, j) -> row map (4096 s + 32 p + j)
    is lexicographic, so host-side reshape is free.
"""

import numpy as np

B = 262144
M = 20
MP = 32                    # m padded to 32 for row-group alignment
F = 512
NCORES = 8
BL = B // NCORES           # 32768 rows per core
SLABS = 16
SLAB_ROWS = BL // SLABS    # 2048
CHUNKS = SLAB_ROWS // 128  # 16 chunks of 128 rows per slab
GROUP = 4                  # chunks per transpose group (4*32 = 128 partitions)
NG = CHUNKS // GROUP       # 8 groups per slab

_CACHE = {}


def _build_nc():
    import concourse.bass as bass
    import concourse.tile as tile
    from concourse import bacc, mybir

    f32 = mybir.dt.float32
    f16 = mybir.dt.float16
    COPY = mybir.ActivationFunctionType.Copy

    nc = bacc.Bacc(None, target_bir_lowering=False, debug=False)
    xh = nc.dram_tensor("xh", [SLABS, 128, CHUNKS * MP], f16, kind="ExternalInput")
    xl = nc.dram_tensor("xl", [SLABS, 128, CHUNKS * MP], f16, kind="ExternalInput")
    qh = nc.dram_tensor("qh", [128, F], f16, kind="ExternalInput")
    ql = nc.dram_tensor("ql", [128, F], f16, kind="ExternalInput")
    ident = nc.dram_tensor("ident", [128, 128], f16, kind="ExternalInput")
    out = nc.dram_tensor("out", [SLABS, 128, CHUNKS * F], f32, kind="ExternalOutput")

    with tile.TileContext(nc) as tc:
        with (
            tc.tile_pool(name="const", bufs=1) as cpool,
            tc.tile_pool(name="xin", bufs=SLABS) as xin_pool,
            tc.tile_pool(name="osl", bufs=4) as out_pool,
            tc.tile_pool(name="tt", bufs=4) as tt_pool,
            tc.tile_pool(name="pst", bufs=2, space=bass.MemorySpace.PSUM) as pst_pool,
            tc.tile_pool(name="pso", bufs=6, space=bass.MemorySpace.PSUM) as pso_pool,
        ):
            qh_t = cpool.tile([128, F], f16, tag="qh")
            ql_t = cpool.tile([128, F], f16, tag="ql")
            id_t = cpool.tile([128, 128], f16, tag="id")
            nc.sync.dma_start(qh_t[:], qh[:])
            nc.sync.dma_start(ql_t[:], ql[:])
            nc.sync.dma_start(id_t[:], ident[:])

            # software pipeline: waves+copies for group g are emitted while
            # group g+1's transposes run, so PE never stalls on the DVE copy.
            pending = None

            def emit_group(t, os_tile, g, flush, uid):
                pos = [
                    pso_pool.tile([128, F], f32, name=f"po_{uid}_{k}", tag="po")
                    for k in range(GROUP)
                ]
                # term-major waves: consecutive MMs hit different row groups
                # and stream concurrently through the PE array.
                for term in range(3):
                    for k in range(GROUP):
                        sl = slice(32 * k, 32 * k + 32)
                        hi = t[sl, 0:128]
                        lo = t[sl, 128:256]
                        lhs, rhs = [
                            (hi, qh_t[sl, :]),
                            (hi, ql_t[sl, :]),
                            (lo, qh_t[sl, :]),
                        ][term]
                        nc.tensor.matmul(
                            pos[k][:], lhs, rhs,
                            start=(term == 0), stop=(term == 2),
                            tile_position=(32 * k, 0),
                        )
                for k in range(GROUP):
                    j = g * GROUP + k
                    dst = os_tile[:, j * F:(j + 1) * F]
                    if k % 2 == 0:
                        nc.vector.tensor_copy(dst, pos[k][:])
                    else:
                        nc.scalar.activation(dst, pos[k][:], COPY)
                if flush is not None:
                    fs, ca, cb = flush
                    nc.sync.dma_start(out[fs][:, ca:cb], os_tile[:, ca:cb])

            for s in range(SLABS):
                xh_s = xin_pool.tile([128, CHUNKS * MP], f16, tag="xh_s")
                xl_s = xin_pool.tile([128, CHUNKS * MP], f16, tag="xl_s")
                nc.scalar.dma_start(xh_s[:], xh[s])
                nc.scalar.dma_start(xl_s[:], xl[s])
                os_tile = out_pool.tile([128, CHUNKS * F], f32, name=f"os_{s}", tag="os")
                # flush granularity: 1 group for slab 0, 2 for slab 1,
                # whole slab otherwise (ramp vs DMA-efficiency trade-off)
                fe = 1 if s == 0 else (2 if s == 1 else NG)
                for g in range(NG):
                    pt = pst_pool.tile([128, 256], f16, name=f"pt_{s}_{g}", tag="pt")
                    csl = slice(g * 128, (g + 1) * 128)
                    nc.tensor.transpose(pt[:, 0:128], xh_s[:, csl], id_t[:])
                    nc.tensor.transpose(pt[:, 128:256], xl_s[:, csl], id_t[:])
                    t = tt_pool.tile([128, 256], f16, name=f"t_{s}_{g}", tag="t")
                    nc.vector.tensor_copy(t[:], pt[:])
                    if pending is not None:
                        emit_group(*pending)
                    flush = None
                    if (g + 1) % fe == 0:
                        flush = (s, (g + 1 - fe) * GROUP * F, (g + 1) * GROUP * F)
                    pending = (t, os_tile, g, flush, f"{s}_{g}")
            emit_group(*pending)

    nc.compile()
    return nc


def _get_nc():
    if "nc" not in _CACHE:
        _CACHE["nc"] = _build_nc()
    return _CACHE["nc"]


def _split_f16(a):
    hi = a.astype(np.float16)
    lo = (a - hi.astype(np.float32)).astype(np.float16)
    return hi, lo


def _prep_inputs(input, weight):
    w = weight.astype(np.float32) + np.float32(1e-8)
    q, _ = np.linalg.qr(w)                      # reduced: (512, 20)
    qt = np.ascontiguousarray(q.T.astype(np.float32))  # (20, 512)
    qpad = np.zeros((MP, F), dtype=np.float32)
    qpad[:M] = qt
    qh16, ql16 = _split_f16(qpad)
    qh_rep = np.ascontiguousarray(np.tile(qh16, (GROUP, 1)))
    ql_rep = np.ascontiguousarray(np.tile(ql16, (GROUP, 1)))
    ident = np.eye(128, dtype=np.float16)

    x = np.ascontiguousarray(input.astype(np.float32))
    xh16, xl16 = _split_f16(x)
    xph = np.zeros((B, MP), dtype=np.float16)
    xph[:, :M] = xh16
    xpl = np.zeros((B, MP), dtype=np.float16)
    xpl[:, :M] = xl16
    xph = xph.reshape(NCORES, SLABS, 128, CHUNKS * MP)
    xpl = xpl.reshape(NCORES, SLABS, 128, CHUNKS * MP)
    return [
        {
            "xh": np.ascontiguousarray(xph[c]),
            "xl": np.ascontiguousarray(xpl[c]),
            "qh": qh_rep,
            "ql": ql_rep,
            "ident": ident,
        }
        for c in range(NCORES)
    ]


def _run(input, weight, trace=False):
    from concourse.bass_utils import run_bass_kernel_spmd

    nc = _get_nc()
    in_maps = _prep_inputs(input, weight)
    res = run_bass_kernel_spmd(nc, in_maps, list(range(NCORES)), trace=trace)
    parts = [r["out"].reshape(BL, F) for r in res.results]
    full = np.concatenate(parts, axis=0)
    return full, res


def kernel(input, weight):
    out, _ = _run(input, weight, trace=False)
    return out


# revision 12
# speedup vs baseline: 1.1505x; 1.0021x over previous
"""Trainium2 Bass kernel for nn_Direction: out = input @ qr(weight + 1e-8).Q^T.

input: (262144, 20) fp32, weight: (512, 20) fp32 -> out: (262144, 512) fp32.

Strategy (data-parallel over batch, 8 cores; memory-bound target):
  - Host: QR of the tiny 512x20 weight (LAPACK). Q^T and the input are each
    split into an fp16 hi/lo pair (hi = fp16(v), lo = fp16(v - hi)); the
    matmul is computed as hi@qhi + hi@qlo + lo@qhi on the PE in three fp16
    passes accumulated in fp32 PSUM (~2e-7 rel err, 4x faster than fp32
    matmuls which microcode as two half-rate LOW/HIGH passes).
  - Device per core (32768 rows): 8 slabs of 4096 rows. Rows are padded
    m: 20->32 on host so a single 128x128 PE transpose yields four 32-row
    "row groups"; the three fp16 passes are emitted as term-major waves of
    four row-tiled matmuls (tile_position=(32k,0)) that stream concurrently
    through the PE array.
  - PSUM results are copied (DVE/ACT alternating) into a [128, 32*512] fp32
    SBUF slab whose flat layout equals 4096 consecutive output rows, flushed
    as one contiguous 8MB DMA. The (s, p, j) -> row map (4096 s + 32 p + j)
    is lexicographic, so host-side reshape is free.
"""

import numpy as np

B = 262144
M = 20
MP = 32                    # m padded to 32 for row-group alignment
F = 512
NCORES = 8
BL = B // NCORES           # 32768 rows per core
SLABS = 16
SLAB_ROWS = BL // SLABS    # 2048
CHUNKS = SLAB_ROWS // 128  # 16 chunks of 128 rows per slab
GROUP = 4                  # chunks per transpose group (4*32 = 128 partitions)
NG = CHUNKS // GROUP       # 8 groups per slab

_CACHE = {}


def _build_nc():
    import concourse.bass as bass
    import concourse.tile as tile
    from concourse import bacc, mybir

    f32 = mybir.dt.float32
    f16 = mybir.dt.float16
    COPY = mybir.ActivationFunctionType.Copy

    nc = bacc.Bacc(None, target_bir_lowering=False, debug=False)
    xh = nc.dram_tensor("xh", [SLABS, 128, CHUNKS * MP], f16, kind="ExternalInput")
    xl = nc.dram_tensor("xl", [SLABS, 128, CHUNKS * MP], f16, kind="ExternalInput")
    qh = nc.dram_tensor("qh", [128, F], f16, kind="ExternalInput")
    ql = nc.dram_tensor("ql", [128, F], f16, kind="ExternalInput")
    ident = nc.dram_tensor("ident", [128, 128], f16, kind="ExternalInput")
    out = nc.dram_tensor("out", [SLABS, 128, CHUNKS * F], f32, kind="ExternalOutput")

    with tile.TileContext(nc) as tc:
        with (
            tc.tile_pool(name="const", bufs=1) as cpool,
            tc.tile_pool(name="xin", bufs=SLABS) as xin_pool,
            tc.tile_pool(name="osl", bufs=4) as out_pool,
            tc.tile_pool(name="tt", bufs=4) as tt_pool,
            tc.tile_pool(name="pst", bufs=2, space=bass.MemorySpace.PSUM) as pst_pool,
            tc.tile_pool(name="pso", bufs=6, space=bass.MemorySpace.PSUM) as pso_pool,
        ):
            qh_t = cpool.tile([128, F], f16, tag="qh")
            ql_t = cpool.tile([128, F], f16, tag="ql")
            id_t = cpool.tile([128, 128], f16, tag="id")
            nc.sync.dma_start(qh_t[:], qh[:])
            nc.sync.dma_start(ql_t[:], ql[:])
            nc.sync.dma_start(id_t[:], ident[:])

            # software pipeline: waves+copies for group g are emitted while
            # group g+1's transposes run, so PE never stalls on the DVE copy.
            pending = None

            def emit_group(t, os_tile, g, flush, uid):
                pos = [
                    pso_pool.tile([128, F], f32, name=f"po_{uid}_{k}", tag="po")
                    for k in range(GROUP)
                ]
                # term-major waves: consecutive MMs hit different row groups
                # and stream concurrently through the PE array.
                for term in range(3):
                    for k in range(GROUP):
                        sl = slice(32 * k, 32 * k + 32)
                        hi = t[sl, 0:128]
                        lo = t[sl, 128:256]
                        lhs, rhs = [
                            (hi, qh_t[sl, :]),
                            (hi, ql_t[sl, :]),
                            (lo, qh_t[sl, :]),
                        ][term]
                        nc.tensor.matmul(
                            pos[k][:], lhs, rhs,
                            start=(term == 0), stop=(term == 2),
                            tile_position=(32 * k, 0),
                        )
                for k in range(GROUP):
                    j = g * GROUP + k
                    dst = os_tile[:, j * F:(j + 1) * F]
                    if k % 2 == 0:
                        nc.vector.tensor_copy(dst, pos[k][:])
                    else:
                        nc.scalar.activation(dst, pos[k][:], COPY)
                if flush is not None:
                    fs, ca, cb = flush
                    nc.sync.dma_start(out[fs][:, ca:cb], os_tile[:, ca:cb])

            for s in range(SLABS):
                xh_s = xin_pool.tile([128, CHUNKS * MP], f16, tag="xh_s")
                xl_s = xin_pool.tile([128, CHUNKS * MP], f16, tag="xl_s")
                nc.scalar.dma_start(xh_s[:], xh[s])
                nc.scalar.dma_start(xl_s[:], xl[s])
                os_tile = out_pool.tile([128, CHUNKS * F], f32, name=f"os_{s}", tag="os")
                # flush granularity: fine at the head (earlier first flush)
                # and at the tail (shorter final drain), coarse in between
                if s == 0 or s == SLABS - 1:
                    fe = 1
                elif s == 1 or s == SLABS - 2:
                    fe = 2
                else:
                    fe = NG
                for g in range(NG):
                    pt = pst_pool.tile([128, 256], f16, name=f"pt_{s}_{g}", tag="pt")
                    csl = slice(g * 128, (g + 1) * 128)
                    nc.tensor.transpose(pt[:, 0:128], xh_s[:, csl], id_t[:])
                    nc.tensor.transpose(pt[:, 128:256], xl_s[:, csl], id_t[:])
                    t = tt_pool.tile([128, 256], f16, name=f"t_{s}_{g}", tag="t")
                    nc.vector.tensor_copy(t[:], pt[:])
                    if pending is not None:
                        emit_group(*pending)
                    flush = None
                    if (g + 1) % fe == 0:
                        flush = (s, (g + 1 - fe) * GROUP * F, (g + 1) * GROUP * F)
                    pending = (t, os_tile, g, flush, f"{s}_{g}")
            emit_group(*pending)

    nc.compile()
    return nc


def _get_nc():
    if "nc" not in _CACHE:
        _CACHE["nc"] = _build_nc()
    return _CACHE["nc"]


def _split_f16(a):
    hi = a.astype(np.float16)
    lo = (a - hi.astype(np.float32)).astype(np.float16)
    return hi, lo


def _prep_inputs(input, weight):
    w = weight.astype(np.float32) + np.float32(1e-8)
    q, _ = np.linalg.qr(w)                      # reduced: (512, 20)
    qt = np.ascontiguousarray(q.T.astype(np.float32))  # (20, 512)
    qpad = np.zeros((MP, F), dtype=np.float32)
    qpad[:M] = qt
    qh16, ql16 = _split_f16(qpad)
    qh_rep = np.ascontiguousarray(np.tile(qh16, (GROUP, 1)))
    ql_rep = np.ascontiguousarray(np.tile(ql16, (GROUP, 1)))
    ident = np.eye(128, dtype=np.float16)

    x = np.ascontiguousarray(input.astype(np.float32))
    xh16, xl16 = _split_f16(x)
    xph = np.zeros((B, MP), dtype=np.float16)
    xph[:, :M] = xh16
    xpl = np.zeros((B, MP), dtype=np.float16)
    xpl[:, :M] = xl16
    xph = xph.reshape(NCORES, SLABS, 128, CHUNKS * MP)
    xpl = xpl.reshape(NCORES, SLABS, 128, CHUNKS * MP)
    return [
        {
            "xh": np.ascontiguousarray(xph[c]),
            "xl": np.ascontiguousarray(xpl[c]),
            "qh": qh_rep,
            "ql": ql_rep,
            "ident": ident,
        }
        for c in range(NCORES)
    ]


def _run(input, weight, trace=False):
    from concourse.bass_utils import run_bass_kernel_spmd

    nc = _get_nc()
    in_maps = _prep_inputs(input, weight)
    res = run_bass_kernel_spmd(nc, in_maps, list(range(NCORES)), trace=trace)
    parts = [r["out"].reshape(BL, F) for r in res.results]
    full = np.concatenate(parts, axis=0)
    return full, res


def kernel(input, weight):
    out, _ = _run(input, weight, trace=False)
    return out
